# revision 26
# baseline (speedup 1.0000x reference)
"""CapsuleLayer kernel for Trainium2, 8 NeuronCores.

Math: the reference's softmax is over a singleton axis, so c_ij == 1 and the
routing loop is dead code.  The output is exactly

    s[b, j, k]  = sum_{i, u} W[0, i, j, k, u] * x[b, u, i]
    m[b, k]     = sum_j s[b, j, k]^2
    v[b, j, k]  = (sqrt(m) / (1 + m)) * s[b, j, k]        (squash)

i.e. one (32 x 32768) @ (32768 x 1024) matmul plus a tiny per-(b,k)
epilogue.  W dominates: the kernel is HBM-bound on reading W once.

Sharding: the output column grid is (k, j) with k = unit_size (64).  Shard on
k: core c owns k in [8c, 8c+8).  Each core reads its W slice (read exactly
once machine-wide), the full x (replicated), and computes a fully local
squash (the j-reduction inside m is intact per core).  Zero cross-core
communication.

Numerics: operands are SINGLE bf16 (not hi/lo pairs).  Measured end metric
(max-abs-err / absmax) is ~4e-3 against the 2e-2 gate: the contraction error
is ~eps_bf16 relative and the squash output is scale-invariant in s.  This
halves HBM traffic vs an fp32-grade hi/lo scheme - per core ~8.4 MB W +
~2.1 MB x - and halves PE streaming time.

Each contraction k-tile is one matmul: stationary x [128, 32], moving W
[128, 128].  Alternate k-tiles go to the two PE column-group halves
(tile_position), so one half's LDWEIGHTS overlaps the other half's stream:
~53 ns/tile vs ~79 serial.  PSUM is [64, 128] (one 32-row block per half);
a tiny bf16 fold matmul sums the halves before the squash.

Host-side, W and x are fused into one streaming tensor (per k-tile
[w 128 cols | x 32 cols]) so a single DMA per chunk carries both operands
with 20 KB contiguous rows; all chunks are SBUF-resident and issued up
front on one queue, so the stream never stalls on buffer reuse and the
per-transfer descriptor rows stay large.
"""

import numpy as np

B, U, I, J, K = 32, 16, 2048, 16, 64  # batch, in_units, in_ch, num_units, unit_size
NC = 8                                # cores
KPC = K // NC                         # unit_size columns per core (8)
N = KPC * J                           # output columns per core (128), kk-major, j-minor
KK = I * U                            # contraction length (32768)
P = 128                               # partitions
KT = KK // P                          # contraction tiles (256)
# Chunk sizes (in contraction tiles): small first chunks so the PE starts
# right away instead of waiting on a big transfer; big middle chunks so DMA
# descriptors are 16 KB/partition-row (measured ~420 GB/s vs ~300 GB/s at
# 8 KB); small last chunks so the PE tail after the final DMA is short.
# All chunks (the whole 8.25 MB W slice) are resident in SBUF at once, so
# every DMA is issued up front and the stream never stalls on buffer reuse.
CHUNKS = [64, 64, 64, 32, 16, 8, 8]
assert sum(CHUNKS) == KT
# x is split in two pieces, interleaved into the W DMA queue order so piece
# 2 lands before the PE needs k-tile 64 (chunks are processed in queue
# order on the issuing engine).
XSPLIT = 64

_CACHE = {}

DEFAULT_CFG = dict(chunks=None, warm_sqrt=True, split=True, fuse_x=True,
                   dual_queue=True, out_eng="sync")


def _build(chunks=None, warm_sqrt=True, split=False, xpieces=2, fuse_x=False,
           dual_queue=False, out_eng="sync"):
    import concourse.bacc as bacc
    import concourse.tile as tile
    import concourse.mybir as mybir

    import concourse.bass as bass

    if chunks is None:
        chunks = CHUNKS
    assert sum(chunks) == KT

    f32 = mybir.dt.float32
    bf16 = mybir.dt.bfloat16
    NB = N + B  # fused per-tile column count (w cols then x cols)
    nc = bacc.Bacc("TRN2", num_devices=NC, debug=False, enable_asserts=False)
    if fuse_x:
        # fused stream: per k-tile [128, 160] bf16 = [w 128 cols | x 32 cols]
        wx_d = nc.dram_tensor("wx", (P, KT * NB), bf16, kind="ExternalInput")
        x_d = w_d = None
    else:
        # x: per k-tile [128, 32] bf16 columns, k-tile-major
        x_d = nc.dram_tensor("x", (P, KT * B), bf16, kind="ExternalInput")
        # w: per k-tile [128, 128] bf16 columns, k-tile-major
        w_d = nc.dram_tensor("w", (P, KT * N), bf16, kind="ExternalInput")
    f_d = None
    if split:
        # fold matrix: f[p, b] = (p % B == b), folds the two PE column
        # groups' PSUM partition blocks with one matmul
        f_d = nc.dram_tensor("f", (2 * B, B), bf16, kind="ExternalInput")
    v_d = nc.dram_tensor("v", (B, KPC, J), f32, kind="ExternalOutput")

    # x piece boundaries (in k-tiles): each piece is enqueued just before
    # the first w chunk that needs it, so the first matmul starts early and
    # the big x bytes never delay the w stream more than necessary.
    if xpieces == 4:
        xb = [0, 16, 48, 176, KT]
    else:
        xb = [0, 64, KT]
    import bisect

    wstart = [0]
    for ch in chunks:
        wstart.append(wstart[-1] + ch)

    # per-size pools with bufs == number of chunks of that size, so no
    # buffer is ever reused and every DMA can issue up front
    from collections import Counter as _Counter

    size_counts = _Counter(chunks)

    MP = 2 * B if split else B
    CW = NB if fuse_x else N  # columns per k-tile in the streamed tensor

    import contextlib

    with tile.TileContext(nc) as tc:
        with contextlib.ExitStack() as stack:
            wpools = {
                sz: stack.enter_context(
                    tc.tile_pool(name=f"w{sz}", bufs=cnt)
                )
                for sz, cnt in size_counts.items()
            }
            if not fuse_x:
                xpool = stack.enter_context(
                    tc.tile_pool(name="xp", bufs=len(xb) - 1)
                )
            ep = stack.enter_context(tc.tile_pool(name="ep", bufs=12))
            ps = stack.enter_context(tc.tile_pool(name="ps", bufs=2, space="PSUM"))

            s_ps = ps.tile([MP, KPC, J], f32)
            f_sb = None
            if split:
                f_sb = ep.tile([MP, B], bf16)
                nc.sync.dma_start(f_sb[:], f_d[:])

            # Issue every DMA up front, alternating between the sync and
            # scalar engines' queues so per-transfer DGE setup gaps on one
            # queue hide behind the other queue's transfers.
            x_sbs = [None] * (len(xb) - 1)
            w_sbs = []
            xi = 0
            dq = [nc.sync, nc.scalar] if dual_queue else [nc.sync]
            for ci, ch in enumerate(chunks):
                if not fuse_x:
                    while xi < len(xb) - 1 and xb[xi] <= wstart[ci]:
                        x_sb = xpool.tile([P, (xb[xi + 1] - xb[xi]) * B], bf16)
                        nc.sync.dma_start(
                            x_sb[:], x_d[:, xb[xi] * B : xb[xi + 1] * B]
                        )
                        x_sbs[xi] = x_sb
                        xi += 1
                w_sb = wpools[ch].tile([P, ch * CW], bf16)
                w_sbs.append(w_sb)
                src_d = wx_d if fuse_x else w_d
                dq[ci % len(dq)].dma_start(
                    w_sb[:], src_d[:, wstart[ci] * CW : wstart[ci + 1] * CW]
                )

            if warm_sqrt:
                # load the ACT sqrt table during the DMA phase (after the
                # scalar engine has issued its DMA configs), not in the
                # serial epilogue
                wtile = ep.tile([1, 1], f32)
                nc.vector.memset(wtile[:], 1.0)
                nc.scalar.sqrt(wtile[:], wtile[:])

            seen = [False, False]
            kt0 = 0
            for ci, ch in enumerate(chunks):
                w_sb = w_sbs[ci]
                for t in range(ch):
                    kt = kt0 + t
                    if fuse_x:
                        lhs = w_sb[:, t * NB + N : (t + 1) * NB]
                        rhs = w_sb[:, t * NB : t * NB + N]
                    else:
                        pi = bisect.bisect_right(xb, kt) - 1
                        lhs = x_sbs[pi][:, (kt - xb[pi]) * B : (kt - xb[pi] + 1) * B]
                        rhs = w_sb[:, t * N : (t + 1) * N]
                    if split:
                        par = kt & 1
                        nc.tensor.matmul(
                            s_ps[par * B : (par + 1) * B],
                            lhs,
                            rhs,
                            start=not seen[par],
                            stop=(kt >= KT - 2),
                            tile_position=(0, par * B),
                            skip_group_check=True,
                        )
                        seen[par] = True
                    else:
                        nc.tensor.matmul(
                            s_ps[:, :, :],
                            lhs,
                            rhs,
                            start=(kt == 0),
                            stop=(kt == KT - 1),
                        )
                kt0 += ch

            # epilogue: (fold the two PE column groups if split) + squash
            if split:
                cp = ep.tile([MP, KPC, J], bf16)
                nc.vector.tensor_copy(cp[:], s_ps[:])
                s2_ps = ps.tile([B, KPC, J], f32)
                nc.tensor.matmul(s2_ps[:], f_sb[:], cp[:], start=True, stop=True)
                s_src = s2_ps
            else:
                s_src = s_ps
            s2 = ep.tile([B, KPC, J], f32)
            nc.scalar.square(s2[:], s_src[:])
            m = ep.tile([B, KPC], f32)
            nc.vector.reduce_sum(m[:], s2[:], axis=mybir.AxisListType.X)
            sq = ep.tile([B, KPC], f32)
            nc.scalar.sqrt(sq[:], m[:])
            d = ep.tile([B, KPC], f32)
            nc.vector.tensor_scalar_add(d[:], m[:], 1.0)
            r = ep.tile([B, KPC], f32)
            nc.vector.reciprocal(r[:], d[:])
            sc = ep.tile([B, KPC], f32)
            nc.vector.tensor_mul(sc[:], sq[:], r[:])
            v_sb = ep.tile([B, KPC, J], f32)
            sc_ap = sc[:]
            sc_bc = bass.AP(
                sc_ap.tensor,
                sc_ap.offset,
                [list(sc_ap.ap[0]), list(sc_ap.ap[1]), [0, J]],
            )
            nc.vector.tensor_mul(v_sb[:], s_src[:], sc_bc)
            getattr(nc, out_eng).dma_start(v_d[:], v_sb[:])

    nc.compile()
    return nc


def get_nc(**cfg):
    key = ("nc", tuple(sorted((k, tuple(v) if isinstance(v, list) else v)
                              for k, v in cfg.items())))
    if key not in _CACHE:
        _CACHE[key] = _build(**cfg)
    return _CACHE[key]


def prep_inputs(x, W, cfg=None):
    """Full inputs -> per-core in_maps with the bf16 streaming layouts."""
    import ml_dtypes

    cfg = cfg or {}
    x = np.ascontiguousarray(np.asarray(x, dtype=np.float32))
    W = np.asarray(W, dtype=np.float32)
    assert x.shape == (B, U, I) and W.shape == (1, I, J, K, U)

    extra = {}
    if cfg.get("split"):
        f = np.zeros((2 * B, B), dtype=np.float32)
        f[np.arange(2 * B), np.arange(2 * B) % B] = 1.0
        extra["f"] = f.astype(ml_dtypes.bfloat16)

    # x[b,u,i] -> [KK=(i major, u minor), b] -> bf16 [KT, P, B]
    xm = x.transpose(2, 1, 0).reshape(KT, P, B).astype(ml_dtypes.bfloat16)

    fuse = cfg.get("fuse_x")
    in_maps = []
    W0 = W[0]  # [I, J, K, U]
    for c in range(NC):
        Wc = W0[:, :, c * KPC : (c + 1) * KPC, :]          # [I, J, KPC, U]
        wm = (
            Wc.transpose(0, 3, 2, 1)
            .reshape(KT, P, N)
            .astype(ml_dtypes.bfloat16)
        )
        if fuse:
            # fused stream: per k-tile [w 128 cols | x 32 cols]
            wx = np.concatenate([wm, xm], axis=2)          # [KT, P, N+B]
            wxhost = np.ascontiguousarray(
                wx.transpose(1, 0, 2).reshape(P, KT * (N + B))
            )
            in_maps.append({"wx": wxhost, **extra})
        else:
            xhost = np.ascontiguousarray(
                xm.transpose(1, 0, 2).reshape(P, KT * B)
            )
            whost = np.ascontiguousarray(
                wm.transpose(1, 0, 2).reshape(P, KT * N)
            )
            in_maps.append({"x": xhost, "w": whost, **extra})
    return in_maps


def gather_output(results):
    """Per-core "v" [B, KPC, J] -> full [B, J, K]."""
    out = np.empty((B, J, K), dtype=np.float32)
    for c in range(NC):
        out[:, :, c * KPC : (c + 1) * KPC] = results[c]["v"].transpose(0, 2, 1)
    return out


def run(x, W, cfg=None, in_maps=None, **spmd_kwargs):
    from concourse import bass_utils

    if cfg is None:
        cfg = DEFAULT_CFG
    nc = get_nc(**cfg)
    if in_maps is None:
        in_maps = prep_inputs(x, W, cfg=cfg)
    res = bass_utils.run_bass_kernel_spmd(
        nc, in_maps, core_ids=list(range(NC)), **spmd_kwargs
    )
    return gather_output(res.results), res


def kernel(x, W):
    out, _ = run(x, W)
    return out


# revision 27
# speedup vs baseline: 1.1215x; 1.1215x over previous
"""CapsuleLayer kernel for Trainium2, 8 NeuronCores.

Math: the reference's softmax is over a singleton axis, so c_ij == 1 and the
routing loop is dead code.  The output is exactly

    s[b, j, k]  = sum_{i, u} W[0, i, j, k, u] * x[b, u, i]
    m[b, k]     = sum_j s[b, j, k]^2
    v[b, j, k]  = (sqrt(m) / (1 + m)) * s[b, j, k]        (squash)

i.e. one (32 x 32768) @ (32768 x 1024) matmul plus a tiny per-(b,k)
epilogue.  W dominates: the kernel is HBM-bound on reading W once.

Sharding: the output column grid is (k, j) with k = unit_size (64).  Shard on
k: core c owns k in [8c, 8c+8).  Each core reads its W slice (read exactly
once machine-wide), the full x (replicated), and computes a fully local
squash (the j-reduction inside m is intact per core).  Zero cross-core
communication.

Numerics: operands are SINGLE bf16 (not hi/lo pairs).  Measured end metric
(max-abs-err / absmax) is ~4e-3 against the 2e-2 gate: the contraction error
is ~eps_bf16 relative and the squash output is scale-invariant in s.  This
halves HBM traffic vs an fp32-grade hi/lo scheme - per core ~8.4 MB W +
~2.1 MB x - and halves PE streaming time.

Each contraction k-tile is one matmul: stationary x [128, 32], moving W
[128, 128].  Alternate k-tiles go to the two PE column-group halves
(tile_position), so one half's LDWEIGHTS overlaps the other half's stream:
~53 ns/tile vs ~79 serial.  PSUM is [64, 128] (one 32-row block per half);
a tiny bf16 fold matmul sums the halves before the squash.

Host-side, W and x are fused into one streaming tensor (per k-tile
[w 128 cols | x 32 cols]) so a single DMA per chunk carries both operands
with 20 KB contiguous rows; all chunks are SBUF-resident and issued up
front on one queue, so the stream never stalls on buffer reuse and the
per-transfer descriptor rows stay large.
"""

import numpy as np

B, U, I, J, K = 32, 16, 2048, 16, 64  # batch, in_units, in_ch, num_units, unit_size
NC = 8                                # cores
KPC = K // NC                         # unit_size columns per core (8)
N = KPC * J                           # output columns per core (128), kk-major, j-minor
KK = I * U                            # contraction length (32768)
P = 128                               # partitions
KT = KK // P                          # contraction tiles (256)
# Chunk sizes (in contraction tiles): small first chunks so the PE starts
# right away instead of waiting on a big transfer; big middle chunks so DMA
# descriptors are 16 KB/partition-row (measured ~420 GB/s vs ~300 GB/s at
# 8 KB); small last chunks so the PE tail after the final DMA is short.
# All chunks (the whole 8.25 MB W slice) are resident in SBUF at once, so
# every DMA is issued up front and the stream never stalls on buffer reuse.
CHUNKS = [64, 64, 64, 32, 16, 8, 8]
assert sum(CHUNKS) == KT
# x is split in two pieces, interleaved into the W DMA queue order so piece
# 2 lands before the PE needs k-tile 64 (chunks are processed in queue
# order on the issuing engine).
XSPLIT = 64

_CACHE = {}

DEFAULT_CFG = dict(chunks=None, warm_sqrt=True, split=True, fuse_x=True,
                   dual_queue=False, out_eng="sync")


def _build(chunks=None, warm_sqrt=True, split=False, xpieces=2, fuse_x=False,
           dual_queue=False, out_eng="sync"):
    import concourse.bacc as bacc
    import concourse.tile as tile
    import concourse.mybir as mybir

    import concourse.bass as bass

    if chunks is None:
        chunks = CHUNKS
    assert sum(chunks) == KT

    f32 = mybir.dt.float32
    bf16 = mybir.dt.bfloat16
    NB = N + B  # fused per-tile column count (w cols then x cols)
    nc = bacc.Bacc("TRN2", num_devices=NC, debug=False, enable_asserts=False)
    if fuse_x:
        # fused stream: per k-tile [128, 160] bf16 = [w 128 cols | x 32 cols]
        wx_d = nc.dram_tensor("wx", (P, KT * NB), bf16, kind="ExternalInput")
        x_d = w_d = None
    else:
        # x: per k-tile [128, 32] bf16 columns, k-tile-major
        x_d = nc.dram_tensor("x", (P, KT * B), bf16, kind="ExternalInput")
        # w: per k-tile [128, 128] bf16 columns, k-tile-major
        w_d = nc.dram_tensor("w", (P, KT * N), bf16, kind="ExternalInput")
    f_d = None
    if split:
        # fold matrix: f[p, b] = (p % B == b), folds the two PE column
        # groups' PSUM partition blocks with one matmul
        f_d = nc.dram_tensor("f", (2 * B, B), bf16, kind="ExternalInput")
    v_d = nc.dram_tensor("v", (B, KPC, J), f32, kind="ExternalOutput")

    # x piece boundaries (in k-tiles): each piece is enqueued just before
    # the first w chunk that needs it, so the first matmul starts early and
    # the big x bytes never delay the w stream more than necessary.
    if xpieces == 4:
        xb = [0, 16, 48, 176, KT]
    else:
        xb = [0, 64, KT]
    import bisect

    wstart = [0]
    for ch in chunks:
        wstart.append(wstart[-1] + ch)

    # per-size pools with bufs == number of chunks of that size, so no
    # buffer is ever reused and every DMA can issue up front
    from collections import Counter as _Counter

    size_counts = _Counter(chunks)

    MP = 2 * B if split else B
    CW = NB if fuse_x else N  # columns per k-tile in the streamed tensor

    import contextlib

    with tile.TileContext(nc) as tc:
        with contextlib.ExitStack() as stack:
            wpools = {
                sz: stack.enter_context(
                    tc.tile_pool(name=f"w{sz}", bufs=cnt)
                )
                for sz, cnt in size_counts.items()
            }
            if not fuse_x:
                xpool = stack.enter_context(
                    tc.tile_pool(name="xp", bufs=len(xb) - 1)
                )
            ep = stack.enter_context(tc.tile_pool(name="ep", bufs=12))
            ps = stack.enter_context(tc.tile_pool(name="ps", bufs=2, space="PSUM"))

            s_ps = ps.tile([MP, KPC, J], f32)
            f_sb = None
            if split:
                f_sb = ep.tile([MP, B], bf16)

            # Issue every DMA up front, alternating between the sync and
            # scalar engines' queues so per-transfer DGE setup gaps on one
            # queue hide behind the other queue's transfers.
            x_sbs = [None] * (len(xb) - 1)
            w_sbs = []
            xi = 0
            dq = [nc.sync, nc.scalar] if dual_queue else [nc.sync]
            for ci, ch in enumerate(chunks):
                if not fuse_x:
                    while xi < len(xb) - 1 and xb[xi] <= wstart[ci]:
                        x_sb = xpool.tile([P, (xb[xi + 1] - xb[xi]) * B], bf16)
                        nc.sync.dma_start(
                            x_sb[:], x_d[:, xb[xi] * B : xb[xi + 1] * B]
                        )
                        x_sbs[xi] = x_sb
                        xi += 1
                w_sb = wpools[ch].tile([P, ch * CW], bf16)
                w_sbs.append(w_sb)
                src_d = wx_d if fuse_x else w_d
                dq[ci % len(dq)].dma_start(
                    w_sb[:], src_d[:, wstart[ci] * CW : wstart[ci + 1] * CW]
                )
                if ci == 0 and split:
                    nc.sync.dma_start(f_sb[:], f_d[:])

            if warm_sqrt:
                # load the ACT sqrt table during the DMA phase (after the
                # scalar engine has issued its DMA configs), not in the
                # serial epilogue
                wtile = ep.tile([1, 1], f32)
                nc.vector.memset(wtile[:], 1.0)
                nc.scalar.sqrt(wtile[:], wtile[:])

            seen = [False, False]
            kt0 = 0
            for ci, ch in enumerate(chunks):
                w_sb = w_sbs[ci]
                for t in range(ch):
                    kt = kt0 + t
                    if fuse_x:
                        lhs = w_sb[:, t * NB + N : (t + 1) * NB]
                        rhs = w_sb[:, t * NB : t * NB + N]
                    else:
                        pi = bisect.bisect_right(xb, kt) - 1
                        lhs = x_sbs[pi][:, (kt - xb[pi]) * B : (kt - xb[pi] + 1) * B]
                        rhs = w_sb[:, t * N : (t + 1) * N]
                    if split:
                        par = kt & 1
                        nc.tensor.matmul(
                            s_ps[par * B : (par + 1) * B],
                            lhs,
                            rhs,
                            start=not seen[par],
                            stop=(kt >= KT - 2),
                            tile_position=(0, par * B),
                            skip_group_check=True,
                        )
                        seen[par] = True
                    else:
                        nc.tensor.matmul(
                            s_ps[:, :, :],
                            lhs,
                            rhs,
                            start=(kt == 0),
                            stop=(kt == KT - 1),
                        )
                kt0 += ch

            # epilogue: (fold the two PE column groups if split) + squash
            if split:
                cp = ep.tile([MP, KPC, J], bf16)
                nc.vector.tensor_copy(cp[:], s_ps[:])
                s2_ps = ps.tile([B, KPC, J], f32)
                nc.tensor.matmul(s2_ps[:], f_sb[:], cp[:], start=True, stop=True)
                s_src = s2_ps
            else:
                s_src = s_ps
            s2 = ep.tile([B, KPC, J], f32)
            nc.scalar.square(s2[:], s_src[:])
            m = ep.tile([B, KPC], f32)
            nc.vector.reduce_sum(m[:], s2[:], axis=mybir.AxisListType.X)
            sq = ep.tile([B, KPC], f32)
            nc.scalar.sqrt(sq[:], m[:])
            d = ep.tile([B, KPC], f32)
            nc.vector.tensor_scalar_add(d[:], m[:], 1.0)
            r = ep.tile([B, KPC], f32)
            nc.vector.reciprocal(r[:], d[:])
            sc = ep.tile([B, KPC], f32)
            nc.vector.tensor_mul(sc[:], sq[:], r[:])
            v_sb = ep.tile([B, KPC, J], f32)
            sc_ap = sc[:]
            sc_bc = bass.AP(
                sc_ap.tensor,
                sc_ap.offset,
                [list(sc_ap.ap[0]), list(sc_ap.ap[1]), [0, J]],
            )
            nc.vector.tensor_mul(v_sb[:], s_src[:], sc_bc)
            getattr(nc, out_eng).dma_start(v_d[:], v_sb[:])

    nc.compile()
    return nc


def get_nc(**cfg):
    key = ("nc", tuple(sorted((k, tuple(v) if isinstance(v, list) else v)
                              for k, v in cfg.items())))
    if key not in _CACHE:
        _CACHE[key] = _build(**cfg)
    return _CACHE[key]


def prep_inputs(x, W, cfg=None):
    """Full inputs -> per-core in_maps with the bf16 streaming layouts."""
    import ml_dtypes

    cfg = cfg or {}
    x = np.ascontiguousarray(np.asarray(x, dtype=np.float32))
    W = np.asarray(W, dtype=np.float32)
    assert x.shape == (B, U, I) and W.shape == (1, I, J, K, U)

    extra = {}
    if cfg.get("split"):
        f = np.zeros((2 * B, B), dtype=np.float32)
        f[np.arange(2 * B), np.arange(2 * B) % B] = 1.0
        extra["f"] = f.astype(ml_dtypes.bfloat16)

    # x[b,u,i] -> [KK=(i major, u minor), b] -> bf16 [KT, P, B]
    xm = x.transpose(2, 1, 0).reshape(KT, P, B).astype(ml_dtypes.bfloat16)

    fuse = cfg.get("fuse_x")
    in_maps = []
    W0 = W[0]  # [I, J, K, U]
    for c in range(NC):
        Wc = W0[:, :, c * KPC : (c + 1) * KPC, :]          # [I, J, KPC, U]
        wm = (
            Wc.transpose(0, 3, 2, 1)
            .reshape(KT, P, N)
            .astype(ml_dtypes.bfloat16)
        )
        if fuse:
            # fused stream: per k-tile [w 128 cols | x 32 cols]
            wx = np.concatenate([wm, xm], axis=2)          # [KT, P, N+B]
            wxhost = np.ascontiguousarray(
                wx.transpose(1, 0, 2).reshape(P, KT * (N + B))
            )
            in_maps.append({"wx": wxhost, **extra})
        else:
            xhost = np.ascontiguousarray(
                xm.transpose(1, 0, 2).reshape(P, KT * B)
            )
            whost = np.ascontiguousarray(
                wm.transpose(1, 0, 2).reshape(P, KT * N)
            )
            in_maps.append({"x": xhost, "w": whost, **extra})
    return in_maps


def gather_output(results):
    """Per-core "v" [B, KPC, J] -> full [B, J, K]."""
    out = np.empty((B, J, K), dtype=np.float32)
    for c in range(NC):
        out[:, :, c * KPC : (c + 1) * KPC] = results[c]["v"].transpose(0, 2, 1)
    return out


def run(x, W, cfg=None, in_maps=None, **spmd_kwargs):
    from concourse import bass_utils

    if cfg is None:
        cfg = DEFAULT_CFG
    nc = get_nc(**cfg)
    if in_maps is None:
        in_maps = prep_inputs(x, W, cfg=cfg)
    res = bass_utils.run_bass_kernel_spmd(
        nc, in_maps, core_ids=list(range(NC)), **spmd_kwargs
    )
    return gather_output(res.results), res


def kernel(x, W):
    out, _ = run(x, W)
    return out


# revision 28
# speedup vs baseline: 1.1620x; 1.0361x over previous
"""CapsuleLayer kernel for Trainium2, 8 NeuronCores.

Math: the reference's softmax is over a singleton axis, so c_ij == 1 and the
routing loop is dead code.  The output is exactly

    s[b, j, k]  = sum_{i, u} W[0, i, j, k, u] * x[b, u, i]
    m[b, k]     = sum_j s[b, j, k]^2
    v[b, j, k]  = (sqrt(m) / (1 + m)) * s[b, j, k]        (squash)

i.e. one (32 x 32768) @ (32768 x 1024) matmul plus a tiny per-(b,k)
epilogue.  W dominates: the kernel is HBM-bound on reading W once.

Sharding: the output column grid is (k, j) with k = unit_size (64).  Shard on
k: core c owns k in [8c, 8c+8).  Each core reads its W slice (read exactly
once machine-wide), the full x (replicated), and computes a fully local
squash (the j-reduction inside m is intact per core).  Zero cross-core
communication.

Numerics: operands are SINGLE bf16 (not hi/lo pairs).  Measured end metric
(max-abs-err / absmax) is ~4e-3 against the 2e-2 gate: the contraction error
is ~eps_bf16 relative and the squash output is scale-invariant in s.  This
halves HBM traffic vs an fp32-grade hi/lo scheme - per core ~8.4 MB W +
~2.1 MB x - and halves PE streaming time.

Each contraction k-tile is one matmul: stationary x [128, 32], moving W
[128, 128].  Alternate k-tiles go to the two PE column-group halves
(tile_position), so one half's LDWEIGHTS overlaps the other half's stream:
~53 ns/tile vs ~79 serial.  PSUM is [64, 128] (one 32-row block per half);
a tiny bf16 fold matmul sums the halves before the squash.

Host-side, W and x are fused into one streaming tensor (per k-tile
[w 128 cols | x 32 cols]) so a single DMA per chunk carries both operands
with 20 KB contiguous rows; all chunks are SBUF-resident and issued up
front on one queue, so the stream never stalls on buffer reuse and the
per-transfer descriptor rows stay large.
"""

import numpy as np

B, U, I, J, K = 32, 16, 2048, 16, 64  # batch, in_units, in_ch, num_units, unit_size
NC = 8                                # cores
KPC = K // NC                         # unit_size columns per core (8)
N = KPC * J                           # output columns per core (128), kk-major, j-minor
KK = I * U                            # contraction length (32768)
P = 128                               # partitions
KT = KK // P                          # contraction tiles (256)
# Chunk sizes (in contraction tiles): small first chunks so the PE starts
# right away instead of waiting on a big transfer; big middle chunks so DMA
# descriptors are 16 KB/partition-row (measured ~420 GB/s vs ~300 GB/s at
# 8 KB); small last chunks so the PE tail after the final DMA is short.
# All chunks (the whole 8.25 MB W slice) are resident in SBUF at once, so
# every DMA is issued up front and the stream never stalls on buffer reuse.
CHUNKS = [64, 64, 64, 32, 16, 8, 8]
assert sum(CHUNKS) == KT
# x is split in two pieces, interleaved into the W DMA queue order so piece
# 2 lands before the PE needs k-tile 64 (chunks are processed in queue
# order on the issuing engine).
XSPLIT = 64

_CACHE = {}

DEFAULT_CFG = dict(chunks=None, warm_sqrt=True, split=True, fuse_x=True,
                   dual_queue=False, out_eng="sync")


def _build(chunks=None, warm_sqrt=True, split=False, xpieces=2, fuse_x=False,
           dual_queue=False, out_eng="sync"):
    import concourse.bacc as bacc
    import concourse.tile as tile
    import concourse.mybir as mybir

    import concourse.bass as bass

    if chunks is None:
        chunks = CHUNKS
    assert sum(chunks) == KT

    f32 = mybir.dt.float32
    bf16 = mybir.dt.bfloat16
    NB = N + B  # fused per-tile column count (w cols then x cols)
    nc = bacc.Bacc("TRN2", num_devices=NC, debug=False, enable_asserts=False)
    if fuse_x:
        # fused stream: per k-tile [128, 160] bf16 = [w 128 cols | x 32 cols]
        wx_d = nc.dram_tensor("wx", (P, KT * NB), bf16, kind="ExternalInput")
        x_d = w_d = None
    else:
        # x: per k-tile [128, 32] bf16 columns, k-tile-major
        x_d = nc.dram_tensor("x", (P, KT * B), bf16, kind="ExternalInput")
        # w: per k-tile [128, 128] bf16 columns, k-tile-major
        w_d = nc.dram_tensor("w", (P, KT * N), bf16, kind="ExternalInput")
    f_d = None
    if split:
        # fold matrix: f[p, b] = (p % B == b), folds the two PE column
        # groups' PSUM partition blocks with one matmul
        f_d = nc.dram_tensor("f", (2 * B, B), bf16, kind="ExternalInput")
    v_d = nc.dram_tensor("v", (B, KPC, J), f32, kind="ExternalOutput")

    # x piece boundaries (in k-tiles): each piece is enqueued just before
    # the first w chunk that needs it, so the first matmul starts early and
    # the big x bytes never delay the w stream more than necessary.
    if xpieces == 4:
        xb = [0, 16, 48, 176, KT]
    else:
        xb = [0, 64, KT]
    import bisect

    wstart = [0]
    for ch in chunks:
        wstart.append(wstart[-1] + ch)

    # per-size pools with bufs == number of chunks of that size, so no
    # buffer is ever reused and every DMA can issue up front
    from collections import Counter as _Counter

    size_counts = _Counter(chunks)

    MP = 2 * B if split else B
    CW = NB if fuse_x else N  # columns per k-tile in the streamed tensor

    import contextlib

    with tile.TileContext(nc) as tc:
        with contextlib.ExitStack() as stack:
            wpools = {
                sz: stack.enter_context(
                    tc.tile_pool(name=f"w{sz}", bufs=cnt)
                )
                for sz, cnt in size_counts.items()
            }
            if not fuse_x:
                xpool = stack.enter_context(
                    tc.tile_pool(name="xp", bufs=len(xb) - 1)
                )
            ep = stack.enter_context(tc.tile_pool(name="ep", bufs=12))
            ps = stack.enter_context(tc.tile_pool(name="ps", bufs=2, space="PSUM"))

            s_ps = ps.tile([MP, KPC, J], f32)
            f_sb = None
            if split:
                f_sb = ep.tile([MP, B], bf16)
                nc.sync.dma_start(f_sb[:], f_d[:])

            # Issue every DMA up front, alternating between the sync and
            # scalar engines' queues so per-transfer DGE setup gaps on one
            # queue hide behind the other queue's transfers.
            x_sbs = [None] * (len(xb) - 1)
            w_sbs = []
            xi = 0
            dq = [nc.sync, nc.scalar] if dual_queue else [nc.sync]
            for ci, ch in enumerate(chunks):
                if not fuse_x:
                    while xi < len(xb) - 1 and xb[xi] <= wstart[ci]:
                        x_sb = xpool.tile([P, (xb[xi + 1] - xb[xi]) * B], bf16)
                        nc.sync.dma_start(
                            x_sb[:], x_d[:, xb[xi] * B : xb[xi + 1] * B]
                        )
                        x_sbs[xi] = x_sb
                        xi += 1
                w_sb = wpools[ch].tile([P, ch * CW], bf16)
                w_sbs.append(w_sb)
                src_d = wx_d if fuse_x else w_d
                dq[ci % len(dq)].dma_start(
                    w_sb[:], src_d[:, wstart[ci] * CW : wstart[ci + 1] * CW]
                )

            if warm_sqrt:
                # load the ACT sqrt table during the DMA phase (after the
                # scalar engine has issued its DMA configs), not in the
                # serial epilogue
                wtile = ep.tile([1, 1], f32)
                nc.vector.memset(wtile[:], 1.0)
                nc.scalar.sqrt(wtile[:], wtile[:])

            seen = [False, False]
            kt0 = 0
            for ci, ch in enumerate(chunks):
                w_sb = w_sbs[ci]
                for t in range(ch):
                    kt = kt0 + t
                    if fuse_x:
                        lhs = w_sb[:, t * NB + N : (t + 1) * NB]
                        rhs = w_sb[:, t * NB : t * NB + N]
                    else:
                        pi = bisect.bisect_right(xb, kt) - 1
                        lhs = x_sbs[pi][:, (kt - xb[pi]) * B : (kt - xb[pi] + 1) * B]
                        rhs = w_sb[:, t * N : (t + 1) * N]
                    if split:
                        par = kt & 1
                        nc.tensor.matmul(
                            s_ps[par * B : (par + 1) * B],
                            lhs,
                            rhs,
                            start=not seen[par],
                            stop=(kt >= KT - 2),
                            tile_position=(0, par * B),
                            skip_group_check=True,
                        )
                        seen[par] = True
                    else:
                        nc.tensor.matmul(
                            s_ps[:, :, :],
                            lhs,
                            rhs,
                            start=(kt == 0),
                            stop=(kt == KT - 1),
                        )
                kt0 += ch

            # epilogue: (fold the two PE column groups if split) + squash
            if split:
                cp = ep.tile([MP, KPC, J], bf16)
                nc.vector.tensor_copy(cp[:], s_ps[:])
                s2_ps = ps.tile([B, KPC, J], f32)
                nc.tensor.matmul(s2_ps[:], f_sb[:], cp[:], start=True, stop=True)
                s_src = s2_ps
            else:
                s_src = s_ps
            s2 = ep.tile([B, KPC, J], f32)
            nc.scalar.square(s2[:], s_src[:])
            m = ep.tile([B, KPC], f32)
            nc.vector.reduce_sum(m[:], s2[:], axis=mybir.AxisListType.X)
            sq = ep.tile([B, KPC], f32)
            nc.scalar.sqrt(sq[:], m[:])
            d = ep.tile([B, KPC], f32)
            nc.vector.tensor_scalar_add(d[:], m[:], 1.0)
            r = ep.tile([B, KPC], f32)
            nc.vector.reciprocal(r[:], d[:])
            sc = ep.tile([B, KPC], f32)
            nc.vector.tensor_mul(sc[:], sq[:], r[:])
            v_sb = ep.tile([B, KPC, J], f32)
            sc_ap = sc[:]
            sc_bc = bass.AP(
                sc_ap.tensor,
                sc_ap.offset,
                [list(sc_ap.ap[0]), list(sc_ap.ap[1]), [0, J]],
            )
            nc.vector.tensor_mul(v_sb[:], s_src[:], sc_bc)
            getattr(nc, out_eng).dma_start(v_d[:], v_sb[:])

    nc.compile()
    return nc


def get_nc(**cfg):
    key = ("nc", tuple(sorted((k, tuple(v) if isinstance(v, list) else v)
                              for k, v in cfg.items())))
    if key not in _CACHE:
        _CACHE[key] = _build(**cfg)
    return _CACHE[key]


def prep_inputs(x, W, cfg=None):
    """Full inputs -> per-core in_maps with the bf16 streaming layouts."""
    import ml_dtypes

    cfg = cfg or {}
    x = np.ascontiguousarray(np.asarray(x, dtype=np.float32))
    W = np.asarray(W, dtype=np.float32)
    assert x.shape == (B, U, I) and W.shape == (1, I, J, K, U)

    extra = {}
    if cfg.get("split"):
        f = np.zeros((2 * B, B), dtype=np.float32)
        f[np.arange(2 * B), np.arange(2 * B) % B] = 1.0
        extra["f"] = f.astype(ml_dtypes.bfloat16)

    # x[b,u,i] -> [KK=(i major, u minor), b] -> bf16 [KT, P, B]
    xm = x.transpose(2, 1, 0).reshape(KT, P, B).astype(ml_dtypes.bfloat16)

    fuse = cfg.get("fuse_x")
    in_maps = []
    W0 = W[0]  # [I, J, K, U]
    for c in range(NC):
        Wc = W0[:, :, c * KPC : (c + 1) * KPC, :]          # [I, J, KPC, U]
        wm = (
            Wc.transpose(0, 3, 2, 1)
            .reshape(KT, P, N)
            .astype(ml_dtypes.bfloat16)
        )
        if fuse:
            # fused stream: per k-tile [w 128 cols | x 32 cols]
            wx = np.concatenate([wm, xm], axis=2)          # [KT, P, N+B]
            wxhost = np.ascontiguousarray(
                wx.transpose(1, 0, 2).reshape(P, KT * (N + B))
            )
            in_maps.append({"wx": wxhost, **extra})
        else:
            xhost = np.ascontiguousarray(
                xm.transpose(1, 0, 2).reshape(P, KT * B)
            )
            whost = np.ascontiguousarray(
                wm.transpose(1, 0, 2).reshape(P, KT * N)
            )
            in_maps.append({"x": xhost, "w": whost, **extra})
    return in_maps


def gather_output(results):
    """Per-core "v" [B, KPC, J] -> full [B, J, K]."""
    out = np.empty((B, J, K), dtype=np.float32)
    for c in range(NC):
        out[:, :, c * KPC : (c + 1) * KPC] = results[c]["v"].transpose(0, 2, 1)
    return out


def run(x, W, cfg=None, in_maps=None, **spmd_kwargs):
    from concourse import bass_utils

    if cfg is None:
        cfg = DEFAULT_CFG
    nc = get_nc(**cfg)
    if in_maps is None:
        in_maps = prep_inputs(x, W, cfg=cfg)
    res = bass_utils.run_bass_kernel_spmd(
        nc, in_maps, core_ids=list(range(NC)), **spmd_kwargs
    )
    return gather_output(res.results), res


def kernel(x, W):
    out, _ = run(x, W)
    return out
